# revision 6
# baseline (speedup 1.0000x reference)
"""Trainium2 Bass kernel for nn_MetaLearner (meta-learning attention + cosine
prototype scoring), data-parallel over tasks on 8 NeuronCores.

Math (per task):
  c   = [img, txt] @ Wc.T + bc                (Wc = concat(Wi, Wt))
  h   = LN1(c);  q,k,v = h @ W{q,k,v}.T + b   (queries: seqlen=1 -> ctx = v)
  ctx = softmax(q k^T / sqrt(128)) v          (support: seqlen=4)
  f   = LN2(ctx) @ Wo.T + bo
  logits[t,q,c] = 10 * cos(qf[t,q], sf[t,c])

Host-side folds (all linear, exact):
  - LN gains/biases folded into the following projection weights.
  - LN mean-subtraction folded into the producing weights (column-centered
    weights make the projection output zero-mean; attention outputs of
    centered v stay centered since softmax rows sum to 1).
  - 1/sqrt(128) folded into Wq; the x10 cosine scale into the support norms.
  - Inputs are pre-transposed on host so the contraction dim lands on SBUF
    partitions (f32 DMA-transpose is not available on TRN2's xbar).
On-chip layout is "transposed" throughout: activations are [hid, rows].
Matmul operands use float32r (TF32-like, 1 cyc/row); vector/scalar compute
stays float32.
"""
import sys
sys.path.insert(0, "/opt/trn_rl_repo")
import numpy as np

HID = 128
T, Q, S = 256, 64, 4
DI, DTXT = 2048, 768
NCORES = 8
TPC = T // NCORES               # 32 tasks per core
FEAT = DI + DTXT                # 2816
KT = FEAT // 128                # 22 contraction chunks
QROWS = TPC * Q                 # 2048 query rows per core
SROWS = TPC * S                 # 128 support rows per core
ROWS = QROWS + SROWS            # 2176
CHUNKS = [(0, 512), (512, 512), (1024, 512), (1536, 512)]  # query chunks
SCALE_INV = 1.0 / (np.sqrt(HID) + 1e-8)
EPS = 1e-5

_prog = None  # cached compiled Bass program


def _build():
    import concourse.bacc as bacc
    import concourse.tile as tile
    import concourse.mybir as mybir
    import concourse.bass as bass

    F32 = mybir.dt.float32
    F32R = mybir.dt.float32r
    AFT = mybir.ActivationFunctionType
    AX = mybir.AxisListType
    ALU = mybir.AluOpType

    nc = bacc.Bacc()
    xT_d = nc.declare_dram_parameter("xT", [FEAT, ROWS], F32R, isOutput=False)
    wc_d = nc.declare_dram_parameter("wc", [FEAT, HID], F32R, isOutput=False)
    wq_d = nc.declare_dram_parameter("wq", [HID, HID], F32R, isOutput=False)
    wk_d = nc.declare_dram_parameter("wk", [HID, HID], F32R, isOutput=False)
    wv_d = nc.declare_dram_parameter("wv", [HID, HID], F32R, isOutput=False)
    wo_d = nc.declare_dram_parameter("wo", [HID, HID], F32R, isOutput=False)
    bc_d = nc.declare_dram_parameter("bc", [HID], F32, isOutput=False)
    bq_d = nc.declare_dram_parameter("bq", [HID], F32, isOutput=False)
    bk_d = nc.declare_dram_parameter("bk", [HID], F32, isOutput=False)
    bv_d = nc.declare_dram_parameter("bv", [HID], F32, isOutput=False)
    bo_d = nc.declare_dram_parameter("bo", [HID], F32, isOutput=False)
    mask_d = nc.declare_dram_parameter("mask", [SROWS, SROWS], F32, isOutput=False)
    id_d = nc.declare_dram_parameter("ident", [128, 128], F32, isOutput=False)
    on_d = nc.declare_dram_parameter("onesv", [128], F32R, isOutput=False)
    out_d = nc.declare_dram_parameter("logits", [TPC, Q, S], F32, isOutput=True)

    lp = nc.allow_low_precision(reason="float32r tiles are bit-compatible f32")
    lp.__enter__()

    with tile.TileContext(nc) as tc:
        with (
            tc.tile_pool(name="wts", bufs=1) as wts,
            tc.tile_pool(name="qfp", bufs=1) as qfp,
            tc.tile_pool(name="dram", bufs=1, space="DRAM") as dram,
        ):
            # ---- resident constants ----
            wc_t = wts.tile([128, KT, HID], F32R)
            nc.sync.dma_start(out=wc_t, in_=wc_d.rearrange("(c p) m -> p c m", p=128))
            wq_t = wts.tile([128, HID], F32R)
            nc.sync.dma_start(out=wq_t, in_=wq_d[:])
            wk_t = wts.tile([128, HID], F32R)
            nc.sync.dma_start(out=wk_t, in_=wk_d[:])
            wv_t = wts.tile([128, HID], F32R)
            nc.sync.dma_start(out=wv_t, in_=wv_d[:])
            wo_t = wts.tile([128, HID], F32R)
            nc.sync.dma_start(out=wo_t, in_=wo_d[:])
            bc_t = wts.tile([128, 1], F32)
            nc.sync.dma_start(out=bc_t, in_=bc_d[:, None])
            bq_t = wts.tile([128, 1], F32)
            nc.sync.dma_start(out=bq_t, in_=bq_d[:, None])
            bk_t = wts.tile([128, 1], F32)
            nc.sync.dma_start(out=bk_t, in_=bk_d[:, None])
            bv_t = wts.tile([128, 1], F32)
            nc.sync.dma_start(out=bv_t, in_=bv_d[:, None])
            bo_t = wts.tile([128, 1], F32)
            nc.sync.dma_start(out=bo_t, in_=bo_d[:, None])
            mask_t = wts.tile([SROWS, SROWS], F32)
            nc.sync.dma_start(out=mask_t, in_=mask_d[:])
            id_t = wts.tile([128, 128], F32)
            nc.sync.dma_start(out=id_t, in_=id_d[:])
            ones_c = wts.tile([128, 1], F32R)   # ss-reduce lhsT  [K=128, M=1]
            nc.sync.dma_start(out=ones_c, in_=on_d[:, None])
            ones_r = wts.tile([1, 128], F32R)   # broadcast lhsT  [K=1, M=128]
            nc.sync.dma_start(out=ones_r, in_=on_d[None, :])
            eps_t = wts.tile([1, 1], F32)
            nc.vector.memset(eps_t, EPS)

            # persistent outputs of the projection pipeline
            qf_tiles = [qfp.tile([128, 512], F32R, tag=f"qf{n}", name=f"qf{n}")
                        for n in range(4)]
            sf_t = qfp.tile([128, SROWS], F32R, tag="sf")

            nq_scr = dram.tile([QROWS], F32)
            np_scr = dram.tile([SROWS], F32)

            def rstd_from_ss(ss_ps, cn, work, psum, scale):
                """ss PSUM row [1,cn] -> (1/sqrt(ss*scale + EPS)) bcast [128,cn] PSUM."""
                var_r = work.tile([1, 512], F32, tag="var")
                nc.scalar.activation(out=var_r[:, :cn], in_=ss_ps[:, :cn],
                                     func=AFT.Sqrt, bias=eps_t[:], scale=scale)
                rstd_f = work.tile([1, 512], F32, tag="rstdf")
                nc.vector.reciprocal(out=rstd_f[:, :cn], in_=var_r[:, :cn])
                rstd_r = work.tile([1, 512], F32R, tag="rstdr")
                nc.vector.tensor_copy(out=rstd_r[:, :cn], in_=rstd_f[:, :cn])
                R_ps = psum.tile([128, 512], F32, tag="rps", bufs=2)
                nc.tensor.matmul(R_ps[:, :cn], ones_r[:], rstd_r[:, :cn],
                                 start=True, stop=True)
                return R_ps

            def inv_norm_row(ss_ps, cn, work, extra_scale):
                """ss PSUM row -> extra_scale / max(sqrt(ss), 1e-8) in SBUF f32."""
                n_r = work.tile([1, 512], F32, tag="nrm")
                nc.scalar.activation(out=n_r[:, :cn], in_=ss_ps[:, :cn],
                                     func=AFT.Sqrt, bias=0.0, scale=1.0)
                nc.vector.tensor_scalar_max(out=n_r[:, :cn], in0=n_r[:, :cn],
                                            scalar1=1e-8)
                i_r = work.tile([1, 512], F32, tag="inrm")
                nc.vector.reciprocal(out=i_r[:, :cn], in_=n_r[:, :cn])
                if extra_scale != 1.0:
                    nc.vector.tensor_scalar_mul(out=i_r[:, :cn], in0=i_r[:, :cn],
                                                scalar1=extra_scale)
                return i_r

            # ================= phase A: query chunks =================
            with (
                tc.tile_pool(name="xa", bufs=3) as xa,
                tc.tile_pool(name="wka", bufs=2) as wka,
                tc.tile_pool(name="psA", bufs=1, space="PSUM") as psA,
            ):
                for n, (c0, cn) in enumerate(CHUNKS):
                    c_ps = psA.tile([128, 512], F32, tag="cps", bufs=2)
                    for k in range(KT):
                        x_t = xa.tile([128, 512], F32R, tag="x")
                        nc.sync.dma_start(
                            out=x_t[:, :cn],
                            in_=xT_d[k * 128:(k + 1) * 128, c0:c0 + cn])
                        nc.tensor.matmul(c_ps[:, :cn], wc_t[:, k, :], x_t[:, :cn],
                                         start=(k == 0), stop=(k == KT - 1))
                    # evict + bias (already centered via host weight folding)
                    c_f = wka.tile([128, 512], F32, tag="cf")
                    nc.scalar.activation(out=c_f[:, :cn], in_=c_ps[:, :cn],
                                         func=AFT.Identity, bias=bc_t, scale=1.0)
                    # LN1 rstd
                    sq = wka.tile([128, 512], F32R, tag="sq")
                    nc.vector.tensor_mul(out=sq[:, :cn], in0=c_f[:, :cn],
                                         in1=c_f[:, :cn])
                    ss_ps = psA.tile([1, 512], F32, tag="ssps", bufs=2)
                    nc.tensor.matmul(ss_ps[:, :cn], ones_c[:], sq[:, :cn],
                                     start=True, stop=True)
                    R1 = rstd_from_ss(ss_ps, cn, wka, psA, 1.0 / HID)
                    h_t = wka.tile([128, 512], F32R, tag="h")
                    nc.vector.tensor_mul(out=h_t[:, :cn], in0=c_f[:, :cn],
                                         in1=R1[:, :cn])
                    # v projection (queries: ctx == v)
                    v_ps = psA.tile([128, 512], F32, tag="pps", bufs=2)
                    nc.tensor.matmul(v_ps[:, :cn], wv_t[:], h_t[:, :cn],
                                     start=True, stop=True)
                    v_f = wka.tile([128, 512], F32, tag="vf")
                    nc.scalar.activation(out=v_f[:, :cn], in_=v_ps[:, :cn],
                                         func=AFT.Identity, bias=bv_t, scale=1.0)
                    # LN2 rstd
                    sq2 = wka.tile([128, 512], F32R, tag="sq2")
                    nc.vector.tensor_mul(out=sq2[:, :cn], in0=v_f[:, :cn],
                                         in1=v_f[:, :cn])
                    ss2_ps = psA.tile([1, 512], F32, tag="ssps", bufs=2)
                    nc.tensor.matmul(ss2_ps[:, :cn], ones_c[:], sq2[:, :cn],
                                     start=True, stop=True)
                    R2 = rstd_from_ss(ss2_ps, cn, wka, psA, 1.0 / HID)
                    z_t = wka.tile([128, 512], F32R, tag="z")
                    nc.vector.tensor_mul(out=z_t[:, :cn], in0=v_f[:, :cn],
                                         in1=R2[:, :cn])
                    # output projection -> qf (transposed layout, f32r)
                    o_ps = psA.tile([128, 512], F32, tag="pps", bufs=2)
                    nc.tensor.matmul(o_ps[:, :cn], wo_t[:], z_t[:, :cn],
                                     start=True, stop=True)
                    nc.scalar.activation(out=qf_tiles[n][:, :cn], in_=o_ps[:, :cn],
                                         func=AFT.Identity, bias=bo_t, scale=1.0)
                    # query norms -> DRAM scratch
                    sq3 = wka.tile([128, 512], F32R, tag="sq3")
                    nc.vector.tensor_mul(out=sq3[:, :cn], in0=qf_tiles[n][:, :cn],
                                         in1=qf_tiles[n][:, :cn])
                    ss3_ps = psA.tile([1, 512], F32, tag="ssps", bufs=2)
                    nc.tensor.matmul(ss3_ps[:, :cn], ones_c[:], sq3[:, :cn],
                                     start=True, stop=True)
                    inq = inv_norm_row(ss3_ps, cn, wka, 1.0)
                    nc.sync.dma_start(out=nq_scr[c0:c0 + cn][None, :],
                                      in_=inq[:, :cn])

            # ================= phase B: support chunk =================
            with (
                tc.tile_pool(name="xb", bufs=2) as xb,
                tc.tile_pool(name="wkb", bufs=1) as wkb,
                tc.tile_pool(name="psB", bufs=1, space="PSUM") as psB,
            ):
                c0, cn = QROWS, SROWS
                c_ps = psB.tile([128, SROWS], F32, tag="cps")
                for k in range(KT):
                    x_t = xb.tile([128, SROWS], F32R, tag="x")
                    nc.sync.dma_start(
                        out=x_t, in_=xT_d[k * 128:(k + 1) * 128, c0:c0 + cn])
                    nc.tensor.matmul(c_ps[:], wc_t[:, k, :], x_t[:],
                                     start=(k == 0), stop=(k == KT - 1))
                c_f = wkb.tile([128, SROWS], F32, tag="cf")
                nc.scalar.activation(out=c_f, in_=c_ps, func=AFT.Identity,
                                     bias=bc_t, scale=1.0)
                sq = wkb.tile([128, SROWS], F32R, tag="sq")
                nc.vector.tensor_mul(out=sq, in0=c_f, in1=c_f)
                ss_ps = psB.tile([1, SROWS], F32, tag="ssps")
                nc.tensor.matmul(ss_ps[:], ones_c[:], sq[:], start=True, stop=True)
                R1 = rstd_from_ss(ss_ps, cn, wkb, psB, 1.0 / HID)
                h_t = wkb.tile([128, SROWS], F32R, tag="h")
                nc.vector.tensor_mul(out=h_t, in0=c_f, in1=R1[:, :cn])

                # q, k, v projections
                q_ps = psB.tile([128, SROWS], F32, tag="qps")
                nc.tensor.matmul(q_ps[:], wq_t[:], h_t[:], start=True, stop=True)
                qT = wkb.tile([128, SROWS], F32R, tag="qT")
                nc.scalar.activation(out=qT, in_=q_ps, func=AFT.Identity,
                                     bias=bq_t, scale=1.0)
                k_ps = psB.tile([128, SROWS], F32, tag="qps")
                nc.tensor.matmul(k_ps[:], wk_t[:], h_t[:], start=True, stop=True)
                kT = wkb.tile([128, SROWS], F32R, tag="kT")
                nc.scalar.activation(out=kT, in_=k_ps, func=AFT.Identity,
                                     bias=bk_t, scale=1.0)
                v_ps = psB.tile([128, SROWS], F32, tag="qps")
                nc.tensor.matmul(v_ps[:], wv_t[:], h_t[:], start=True, stop=True)
                vT_f = wkb.tile([128, SROWS], F32, tag="vTf")
                nc.scalar.activation(out=vT_f, in_=v_ps, func=AFT.Identity,
                                     bias=bv_t, scale=1.0)

                # scores + masked softmax over the free dim
                s_ps = psB.tile([SROWS, SROWS], F32, tag="sps")
                nc.tensor.matmul(s_ps[:], qT[:], kT[:], start=True, stop=True)
                s_f = wkb.tile([SROWS, SROWS], F32, tag="sf_")
                nc.vector.tensor_add(out=s_f, in0=s_ps, in1=mask_t)
                nmx = wkb.tile([SROWS, 1], F32, tag="nmx")
                nc.vector.tensor_reduce(out=nmx, in_=s_f, axis=AX.X,
                                        op=ALU.max, negate=True)
                a_f = wkb.tile([SROWS, SROWS], F32, tag="af")
                asum = wkb.tile([SROWS, 1], F32, tag="asum")
                nc.scalar.activation(out=a_f, in_=s_f, func=AFT.Exp,
                                     bias=nmx, scale=1.0, accum_out=asum)
                rs = wkb.tile([SROWS, 1], F32, tag="rs")
                nc.vector.reciprocal(out=rs, in_=asum)
                nc.vector.tensor_scalar_mul(out=a_f, in0=a_f, scalar1=rs)

                # ctx^T = v^T a^T via PE transposes
                aT_ps = psB.tile([SROWS, SROWS], F32, tag="tps")
                nc.tensor.matmul(aT_ps[:], a_f[:], id_t[:], is_transpose=True)
                aT = wkb.tile([SROWS, SROWS], F32, tag="aT")
                nc.vector.tensor_copy(out=aT, in_=aT_ps)
                vn_ps = psB.tile([SROWS, SROWS], F32, tag="tps")
                nc.tensor.matmul(vn_ps[:], vT_f[:], id_t[:], is_transpose=True)
                vn = wkb.tile([SROWS, SROWS], F32, tag="vn")
                nc.vector.tensor_copy(out=vn, in_=vn_ps)
                ctx_ps = psB.tile([128, SROWS], F32, tag="cps")
                nc.tensor.matmul(ctx_ps[:], vn[:], aT[:], start=True, stop=True)
                ctx_f = wkb.tile([128, SROWS], F32, tag="ctxf")
                nc.scalar.activation(out=ctx_f, in_=ctx_ps, func=AFT.Copy,
                                     scale=1.0)

                # LN2 (ctx is zero-mean by construction) + output projection
                sq2 = wkb.tile([128, SROWS], F32R, tag="sq2")
                nc.vector.tensor_mul(out=sq2, in0=ctx_f, in1=ctx_f)
                ss2_ps = psB.tile([1, SROWS], F32, tag="ssps")
                nc.tensor.matmul(ss2_ps[:], ones_c[:], sq2[:], start=True, stop=True)
                R2 = rstd_from_ss(ss2_ps, cn, wkb, psB, 1.0 / HID)
                z_t = wkb.tile([128, SROWS], F32R, tag="z")
                nc.vector.tensor_mul(out=z_t, in0=ctx_f, in1=R2[:, :cn])
                o_ps = psB.tile([128, SROWS], F32, tag="qps")
                nc.tensor.matmul(o_ps[:], wo_t[:], z_t[:], start=True, stop=True)
                nc.scalar.activation(out=sf_t, in_=o_ps, func=AFT.Identity,
                                     bias=bo_t, scale=1.0)
                # support norms (x10 cosine scale folded here)
                sq3 = wkb.tile([128, SROWS], F32R, tag="sq3")
                nc.vector.tensor_mul(out=sq3, in0=sf_t, in1=sf_t)
                ss3_ps = psB.tile([1, SROWS], F32, tag="ssps")
                nc.tensor.matmul(ss3_ps[:], ones_c[:], sq3[:], start=True, stop=True)
                inp_r = inv_norm_row(ss3_ps, cn, wkb, 10.0)
                nc.sync.dma_start(out=np_scr[:][None, :], in_=inp_r[:, :cn])

            # ================= phase C: scores + fixup =================
            with (
                tc.tile_pool(name="wkc", bufs=1) as wkc,
                tc.tile_pool(name="psC", bufs=1, space="PSUM") as psC,
            ):
                U_pss = [psC.tile([64, Q], F32, tag=f"ups{h}", name=f"ups{h}")
                         for h in range(2)]
                for t in range(TPC):
                    g = t // 2
                    nc.tensor.matmul(
                        U_pss[t % 2][:, 4 * g:4 * g + 4],
                        qf_tiles[t // 8][:, 64 * (t % 8):64 * (t % 8) + 64],
                        sf_t[:, 4 * t:4 * t + 4],
                        start=True, stop=True)

                import concourse.bass as _b
                nq_base = nq_scr[:]
                np_base = np_scr[:]
                out_base = out_d[:]
                for half in range(2):
                    nq_t = wkc.tile([64, 16], F32, tag=f"nqt{half}",
                                    name=f"nqt{half}")
                    src_nq = _b.AP(tensor=nq_base.tensor,
                                   offset=nq_base.offset + 64 * half,
                                   ap=[[1, 64], [128, 16]])
                    nc.sync.dma_start(out=nq_t, in_=src_nq)
                    npB = wkc.tile([64, Q], F32, tag=f"npb{half}",
                                   name=f"npb{half}")
                    src_np = _b.AP(tensor=np_base.tensor,
                                   offset=np_base.offset + 4 * half,
                                   ap=[[0, 64], [8, 16], [1, 4]])
                    nc.sync.dma_start(
                        out=npB.rearrange("p (g b) -> p g b", b=4), in_=src_np)

                    U_sb = wkc.tile([64, Q], F32, tag=f"usb{half}",
                                    name=f"usb{half}")
                    for g in range(16):
                        nc.vector.tensor_scalar_mul(
                            out=U_sb[:, 4 * g:4 * g + 4],
                            in0=U_pss[half][:, 4 * g:4 * g + 4],
                            scalar1=nq_t[:, g:g + 1])
                    nc.vector.tensor_mul(out=U_sb, in0=U_sb, in1=npB)

                    dst = _b.AP(tensor=out_base.tensor,
                                offset=out_base.offset + 256 * half,
                                ap=[[4, 64], [512, 16], [1, 4]])
                    nc.sync.dma_start(
                        out=dst, in_=U_sb.rearrange("p (g b) -> p g b", b=4))

    lp.__exit__(None, None, None)
    nc.compile()
    return nc


def _host_prep(inputs):
    f32 = np.float32
    Wi, Wt = np.asarray(inputs["Wi"], f32), np.asarray(inputs["Wt"], f32)
    bi, bt = np.asarray(inputs["bi"], f32), np.asarray(inputs["bt"], f32)
    g1, b1 = np.asarray(inputs["g1"], f32), np.asarray(inputs["b1"], f32)
    g2, b2 = np.asarray(inputs["g2"], f32), np.asarray(inputs["b2"], f32)
    Wq, bq = np.asarray(inputs["Wq"], f32), np.asarray(inputs["bq"], f32)
    Wk, bk = np.asarray(inputs["Wk"], f32), np.asarray(inputs["bk"], f32)
    Wv, bv = np.asarray(inputs["Wv"], f32), np.asarray(inputs["bv"], f32)
    Wo, bo = np.asarray(inputs["Wo"], f32), np.asarray(inputs["bo"], f32)

    Wc = np.concatenate([Wi, Wt], axis=1)          # [128, 2816]
    bc = bi + bt
    Wc_c = Wc - Wc.mean(axis=0, keepdims=True)     # fold LN1 mean
    bc_c = bc - bc.mean()

    Wq_f = (Wq * g1[None, :]) * SCALE_INV
    bq_f = (bq + Wq @ b1) * SCALE_INV
    Wk_f = Wk * g1[None, :]
    bk_f = bk + Wk @ b1
    Wv_f = Wv * g1[None, :]
    bv_f = bv + Wv @ b1
    Wv_c = Wv_f - Wv_f.mean(axis=0, keepdims=True)  # fold LN2 mean
    bv_c = bv_f - bv_f.mean()
    Wo_f = Wo * g2[None, :]
    bo_f = bo + Wo @ b2

    blk = np.arange(SROWS) // S
    mask = np.where(blk[:, None] == blk[None, :], 0.0, -1e30).astype(f32)

    common = {
        "wc": np.ascontiguousarray(Wc_c.T), "bc": bc_c,
        "wq": np.ascontiguousarray(Wq_f.T), "bq": bq_f,
        "wk": np.ascontiguousarray(Wk_f.T), "bk": bk_f,
        "wv": np.ascontiguousarray(Wv_c.T), "bv": bv_c,
        "wo": np.ascontiguousarray(Wo_f.T), "bo": bo_f,
        "mask": mask, "ident": np.eye(128, dtype=f32),
        "onesv": np.ones(128, dtype=f32),
    }

    si = np.asarray(inputs["support_images"], f32)
    st = np.asarray(inputs["support_texts"], f32)
    qi = np.asarray(inputs["query_images"], f32)
    qt = np.asarray(inputs["query_texts"], f32)

    in_maps = []
    for m in range(NCORES):
        ts = slice(m * TPC, (m + 1) * TPC)
        Xq = np.concatenate([qi[ts].reshape(QROWS, DI),
                             qt[ts].reshape(QROWS, DTXT)], axis=1)
        Xs = np.concatenate([si[ts].reshape(SROWS, DI),
                             st[ts].reshape(SROWS, DTXT)], axis=1)
        X = np.concatenate([Xq, Xs], axis=0)        # [2176, 2816]
        xT = np.ascontiguousarray(X.T)              # [2816, 2176]
        in_maps.append({"xT": xT, **common})
    return in_maps


def _run(in_maps, trace=False, **kw):
    from concourse.bass_utils import run_bass_kernel_spmd
    global _prog
    if _prog is None:
        _prog = _build()
    return run_bass_kernel_spmd(_prog, in_maps, list(range(NCORES)),
                                trace=trace, **kw)


def kernel(**inputs) -> np.ndarray:
    in_maps = _host_prep(inputs)
    res = _run(in_maps)
    return np.concatenate([res.results[m]["logits"] for m in range(NCORES)],
                          axis=0)
